# revision 1
# baseline (speedup 1.0000x reference)
"""Cost-volume builder (correlation layer) for Trainium2, 8-core SPMD.

out[b, d, h, w] = (1/sqrt(C)) * sum_c feat1[b,c,h,w] * feat2[b,c,h+dy,w+dx]
for d = (dy+4)*9 + (dx+4), dy,dx in [-4,4]. B,C,H,W = 4,128,192,256.

Sharding: 8 cores = 4 batches x 2 H-halves (96 rows each, feat2 halo +-4).

Per-core algorithm (two 48-row halves):
  Pass 1 (PE): for each r-block (8 feat2 rows) x w-tile (8 outputs wide,
    16-wide feat2 window): matmul lhsT=F2win[C,8x16=128] (stationary,
    FWL-eligible) vs rhs=F1[C,16 h-rows x 8 w =128] -> band tile [128,128]
    in PSUM: band[(j,we), (h,w)] = sum_c F2[c,r0+j,we] * F1[c,h,w].
  Stage (DVE/ACT): PSUM->SBUF fp16 cast copies (4 band tiles per bank).
  Pass 2 (PE): 128 constant one-hot selection matrices Sel[128,81]; per
    (phase t, wl) two PSUM-accumulated matmuls over paired r-blocks pick,
    for every output position, its 81 displacement values:
    out2[(dy,dx), (k,w0)] with h = 8k + t - 8, w = 8*w0 + wl.
  Out-copies (DVE/ACT): [81,192] PSUM->SBUF out tile, then DMA to HBM.
"""

import math

import numpy as np

B, C, H, W = 4, 128, 192, 256
D = 81
NCORES = 8
OH = H // 2            # 96 output rows per core
HQ = 48                # rows per processed half
NRB = 7                # r-blocks per half, 8 rows each, r in [-4, 51]
WT = 32                # w-tiles per row (T=8)
T = 8
WE = 16                # feat2 w-window per tile
F2W = W + 8            # 264, zero-padded W
F2H = OH + 8           # 104 rows incl halo
SCALE = 1.0 / math.sqrt(C)


def _build_sel():
    """[128, 128*81] fp16 one-hot selection matrices, class c=(h_off*8+wl).
    Weight-column order of pass-1 lhsT is (kappa, j): row = kappa*8 + j."""
    sel = np.zeros((128, 128, 81), np.float16)
    for h_off in range(16):
        for wl in range(8):
            cls = h_off * 8 + wl
            for j in range(8):
                dy = j + 4 - h_off
                if -4 <= dy <= 4:
                    for dxh in range(9):  # dxh = dx + 4
                        row = (wl + dxh) * 8 + j
                        col = (dy + 4) * 9 + dxh
                        sel[row, cls, col] = 1.0
    return sel.reshape(128, 128 * 81)


def _emit(tc, f1, f2, selt, out):
    """Emit the Tile program. f1:[C,OH*W] f16, f2:[C,F2H*F2W] f16,
    selt:[C,128*81] f16, out:[D,OH*W] f32 (DRAM APs)."""
    import concourse.bass as bass
    import concourse.mybir as mybir

    dt = mybir.dt
    nc = tc.nc
    MS = bass.MemorySpace

    with (
        tc.tile_pool(name="const", bufs=1) as cpool,
        tc.tile_pool(name="f1p", bufs=1) as f1p,
        tc.tile_pool(name="f2p", bufs=3) as f2p,
        tc.tile_pool(name="stgp", bufs=1) as stgp,
        tc.tile_pool(name="outp", bufs=1) as outp,
        tc.tile_pool(name="ps1", bufs=4, space=MS.PSUM) as ps1,
        tc.tile_pool(name="ps2", bufs=4, space=MS.PSUM) as ps2,
    ):
        selb = cpool.tile([128, 128 * 81], dt.float16)
        nc.sync.dma_start(selb[:, :], selt[:, :])

        for q in range(2):
            # ---- load F1 half: rows h in [-8, 55] (halo zeros baked host-side)
            f1h = f1p.tile([128, 64 * W], dt.float16, tag="f1h")
            nc.sync.dma_start(
                f1h[:, :], f1[:, q * 64 * W : (q + 1) * 64 * W]
            )

            # stage layout: col = cls * 224 + (k*32 + w0), cls = h_off*8 + wl
            stg = stgp.tile([128, 128 * NRB * WT], dt.float16, tag="stg")
            stv2 = stg[:, :].rearrange("p (c t) -> p c t", c=128)
            eng = 0

            # ---- pass 1: band matmuls ----
            for k in range(NRB):
                # slab s = k + NRB*q: [C, 264, 8] = F2 rows [8k-4+48q, 8k+3+48q]
                # transposed host-side so each 16x8 window is contiguous.
                f2s = f2p.tile([128, F2W * 8], dt.float16, tag="f2s")
                s = k + NRB * q
                nc.sync.dma_start(
                    f2s[:, :],
                    f2[:, s * F2W * 8 : (s + 1) * F2W * 8],
                )
                f1v = f1h[:, :].rearrange("p (h x) -> p h x", h=64)
                for g in range(8):  # groups of 4 w-tiles per PSUM bank
                    pt = ps1.tile([128, 512], dt.float32, tag="ps1")
                    for u in range(4):
                        w0 = g * 4 + u
                        lhsT = f2s[:, 64 * w0 : 64 * w0 + 128]     # [128,128]
                        rhs = f1v[:, 8 * k : 8 * k + 16, 8 * w0 : 8 * w0 + T]
                        nc.tensor.matmul(
                            pt[:, u * 128 : (u + 1) * 128],
                            lhsT,
                            rhs,
                            start=True,
                            stop=True,
                        )
                    # psum col = u*128 + cls  ->  stage (cls, t0+u)
                    t0 = k * 32 + g * 4
                    src = pt[:, :].rearrange("p (u c) -> p c u", u=4)
                    dst = stv2[:, :, t0 : t0 + 4]
                    if eng == 0:
                        nc.vector.tensor_copy(dst, src)
                    else:
                        nc.scalar.copy(dst, src)
                    eng ^= 1

            # ---- pass 2: selection matmuls + out copies ----
            outh = outp.tile([81, HQ * W], dt.float32, tag="outh")
            ov = outh[:, :].rearrange(
                "p (a b c d) -> p a b c d", a=6, b=8, c=32, d=8
            )  # h = 8a + b, w = 8c + d
            for t in range(8, 16):
                for wl in range(8):
                    clsA = t * 8 + wl
                    clsB = (t - 8) * 8 + wl
                    p2 = ps2.tile([128, 192], dt.float32, tag="ps2")
                    rhsA = stv2[:, clsA, 0:192]   # [128, 192] tiles k=0..5
                    rhsB = stv2[:, clsB, 32:224]  # [128, 192] tiles k=1..6
                    nc.tensor.matmul(
                        p2[0:81, :],
                        selb[:, clsA * 81 : (clsA + 1) * 81],
                        rhsA,
                        start=True,
                        stop=False,
                    )
                    nc.tensor.matmul(
                        p2[0:81, :],
                        selb[:, clsB * 81 : (clsB + 1) * 81],
                        rhsB,
                        start=False,
                        stop=True,
                    )
                    dst = ov[:, :, t - 8, :, wl]  # [81, 6, 32]
                    src = p2[0:81, :].rearrange("p (a b) -> p a b", a=6)
                    if eng == 0:
                        nc.vector.tensor_copy(dst, src)
                    else:
                        nc.scalar.copy(dst, src)
                    eng ^= 1

            nc.sync.dma_start(
                out[:, q * HQ * W : (q + 1) * HQ * W], outh[:, :]
            )


def _build_nc():
    import concourse.mybir as mybir
    import concourse.tile as tile
    from concourse import bacc

    dt = mybir.dt
    nc = bacc.Bacc("TRN2", target_bir_lowering=False, debug=False)
    f1 = nc.dram_tensor("f1", [C, 2 * 64 * W], dt.float16, kind="ExternalInput")
    f2 = nc.dram_tensor(
        "f2", [C, 2 * NRB * F2W * 8], dt.float16, kind="ExternalInput"
    )
    selt = nc.dram_tensor("sel", [C, 128 * 81], dt.float16, kind="ExternalInput")
    out = nc.dram_tensor("out", [D, OH * W], dt.float32, kind="ExternalOutput")
    with tile.TileContext(nc) as tc:
        _emit(tc, f1[:, :], f2[:, :], selt[:, :], out[:, :])
    nc.finalize()
    return nc


def _shard_inputs(feat1, feat2):
    sel = _build_sel()
    in_maps = []
    for core in range(NCORES):
        b, half = core // 2, core % 2
        h0 = half * OH
        f1s = np.zeros((C, 2, 64, W), np.float16)
        for q in range(2):
            glo, ghi = q * HQ - 8, q * HQ + 56   # rows -8..55 of this half
            slo, shi = max(glo, 0), min(ghi, OH)
            f1s[:, q, slo - glo : shi - glo, :] = (
                feat1[b, :, h0 + slo : h0 + shi, :] * SCALE
            ).astype(np.float16)
        f2pad = np.zeros((C, F2H, F2W), np.float16)
        lo, hi = h0 - 4, h0 + OH + 4
        slo, shi = max(lo, 0), min(hi, H)
        f2pad[:, slo - lo : shi - lo, 4 : 4 + W] = feat2[b, :, slo:shi, :].astype(
            np.float16
        )
        # slabs: s = k + NRB*q -> F2 rows [8k+48q, 8k+48q+8), transposed to
        # [C, w, r] so each (16 w x 8 r) matmul weight window is contiguous.
        slabs = np.zeros((C, 2 * NRB, F2W, 8), np.float16)
        for s in range(2 * NRB):
            base = 8 * (s % NRB) + HQ * (s // NRB)
            slabs[:, s] = f2pad[:, base : base + 8, :].transpose(0, 2, 1)
        in_maps.append(
            {
                "f1": np.ascontiguousarray(f1s.reshape(C, 2 * 64 * W)),
                "f2": np.ascontiguousarray(slabs.reshape(C, -1)),
                "sel": sel,
            }
        )
    return in_maps


def kernel(feat1, feat2):
    feat1 = np.asarray(feat1, dtype=np.float32)
    feat2 = np.asarray(feat2, dtype=np.float32)
    from concourse.bass_utils import run_bass_kernel_spmd

    nc = _build_nc()
    in_maps = _shard_inputs(feat1, feat2)
    res = run_bass_kernel_spmd(nc, in_maps, list(range(NCORES)))
    full = np.zeros((B, D, H, W), np.float32)
    for core in range(NCORES):
        b, half = core // 2, core % 2
        full[b, :, half * OH : (half + 1) * OH, :] = res.results[core][
            "out"
        ].reshape(D, OH, W)
    return full



# revision 2
# speedup vs baseline: 1.0512x; 1.0512x over previous
"""Cost-volume builder (correlation layer) for Trainium2, 8-core SPMD.

out[b, d, h, w] = (1/sqrt(C)) * sum_c feat1[b,c,h,w] * feat2[b,c,h+dy,w+dx]
for d = (dy+4)*9 + (dx+4), dy,dx in [-4,4]. B,C,H,W = 4,128,192,256.

Sharding: 8 cores = 4 batches x 2 H-halves (96 rows each, feat2 halo +-4).

Per-core algorithm (two 48-row q-halves sharing one f1 buffer):
  Pass 1 (PE): for each r-block (8 feat2 rows) x w-tile (8 outputs wide,
    16-wide feat2 window): matmul lhsT=F2win[C,8x16=128] (stationary)
    vs rhs=F1[C,16 h-rows x 8 w =128] -> band tile [128,128] in PSUM:
    band[(j,we), (h,w)] = sum_c F2[c,r0+j,we] * F1[c,h,w].
  Stage (DVE/ACT): PSUM->SBUF fp16 cast copies (4 band tiles per bank).
  Pass 2 (PE): 128 constant one-hot selection matrices Sel[128,81]; per
    (phase t, wl) two PSUM-accumulated matmuls over paired r-blocks pick,
    for every output position, its 81 displacement values.
  Out-copies (DVE/ACT): [81,192] PSUM->SBUF fp16, chunk-DMA to HBM per
    (q,t); host reassembles the (q,tt,wl,k,w0) order and casts fp32.
"""

import math

import numpy as np

B, C, H, W = 4, 128, 192, 256
D = 81
NCORES = 8
OH = H // 2            # 96 output rows per core
HQ = 48                # rows per processed half
NRB = 7                # r-blocks per half, 8 rows each
NSLAB = 13             # unique f2 slabs per core (slab 6 shared by halves)
WT = 32                # w-tiles per row (T=8)
T = 8
WE = 16                # feat2 w-window per tile
F2W = W + 8            # 264, zero-padded W
F2H = OH + 8           # 104 rows incl halo
SCALE = 1.0 / math.sqrt(C)


def _build_sel():
    """[128, 128*81] fp16 one-hot selection matrices, class c=(h_off*8+wl).
    Weight-column order of pass-1 lhsT is (kappa, j): row = kappa*8 + j."""
    sel = np.zeros((128, 128, 81), np.float16)
    for h_off in range(16):
        for wl in range(8):
            cls = h_off * 8 + wl
            for j in range(8):
                dy = j + 4 - h_off
                if -4 <= dy <= 4:
                    for dxh in range(9):  # dxh = dx + 4
                        row = (wl + dxh) * 8 + j
                        col = (dy + 4) * 9 + dxh
                        sel[row, cls, col] = 1.0
    return sel.reshape(128, 128 * 81)


def _emit(tc, f1, f2, selt, out):
    """Emit the Tile program. f1:[C,96*W] f16, f2:[C,13*F2W*8] f16,
    selt:[C,128*81] f16, out:[D,2*8*1536] f16 (DRAM APs)."""
    import concourse.bass as bass
    import concourse.mybir as mybir

    dt = mybir.dt
    nc = tc.nc
    MS = bass.MemorySpace

    with (
        tc.tile_pool(name="const", bufs=1) as cpool,
        tc.tile_pool(name="f1p", bufs=1) as f1p,
        tc.tile_pool(name="f2p", bufs=4) as f2p,
        tc.tile_pool(name="stgp", bufs=1) as stgp,
        tc.tile_pool(name="outp", bufs=2) as outp,
        tc.tile_pool(name="ps1", bufs=4, space=MS.PSUM) as ps1,
        tc.tile_pool(name="ps2", bufs=4, space=MS.PSUM) as ps2,
    ):
        # ---- persistent tiles ----
        f1buf = f1p.tile([128, 112 * W], dt.float16)     # rows h in [-8, 104)
        selb = cpool.tile([128, 128 * 81], dt.float16)
        slab6 = cpool.tile([128, F2W * 8], dt.float16)

        # f1 halo memsets (rows -8..0 and 96..104)
        nc.gpsimd.memset(f1buf[:, 0 : 8 * W], 0.0)
        nc.gpsimd.memset(f1buf[:, 104 * W : 112 * W], 0.0)
        # f1 rows 0..96 in 6 chunks of 16 rows (sync HWDGE queue)
        for c in range(6):
            nc.sync.dma_start(
                f1buf[:, (8 + 16 * c) * W : (24 + 16 * c) * W],
                f1[:, 16 * c * W : (16 * c + 16) * W],
            )
        # sel on the scalar HWDGE queue (parallel ring; needed by pass 2)
        nc.scalar.dma_start(selb[:, :], selt[:, :])
        # slab 6 (shared by both halves) early on sync queue
        nc.sync.dma_start(slab6[:, :], f2[:, 6 * F2W * 8 : 7 * F2W * 8])

        f1v = f1buf[:, :].rearrange("p (h x) -> p h x", h=112)
        eng = 0

        for q in range(2):
            # stage layout: col = cls * 224 + (k*32 + w0), cls = h_off*8 + wl
            stg = stgp.tile([128, 128 * NRB * WT], dt.float16, tag="stg")
            stv2 = stg[:, :].rearrange("p (c t) -> p c t", c=128)

            # ---- pass 1: band matmuls ----
            for k in range(NRB):
                s = 6 * q + k
                if s == 6:
                    f2s = slab6
                else:
                    f2s = f2p.tile([128, F2W * 8], dt.float16, tag="f2s")
                    nc.sync.dma_start(
                        f2s[:, :], f2[:, s * F2W * 8 : (s + 1) * F2W * 8]
                    )
                r0 = 48 * q + 8 * k
                for g in range(8):  # groups of 4 w-tiles per PSUM bank
                    pt = ps1.tile([128, 512], dt.float32, tag="ps1")
                    for u in range(4):
                        w0 = g * 4 + u
                        lhsT = f2s[:, 64 * w0 : 64 * w0 + 128]     # [128,128]
                        rhs = f1v[:, r0 : r0 + 16, 8 * w0 : 8 * w0 + T]
                        nc.tensor.matmul(
                            pt[:, u * 128 : (u + 1) * 128],
                            lhsT,
                            rhs,
                            start=True,
                            stop=True,
                        )
                    # psum col = u*128 + cls  ->  stage (cls, t0+u)
                    t0 = k * 32 + g * 4
                    src = pt[:, :].rearrange("p (u c) -> p c u", u=4)
                    dst = stv2[:, :, t0 : t0 + 4]
                    if eng == 0:
                        nc.vector.tensor_copy(dst, src)
                    else:
                        nc.scalar.copy(dst, src)
                    eng ^= 1

            # ---- pass 2: selection matmuls + out copies + chunked stores ----
            for t in range(8, 16):
                outt = outp.tile([81, 8 * 192], dt.float16, tag="outt")
                for wl in range(8):
                    clsA = t * 8 + wl
                    clsB = (t - 8) * 8 + wl
                    p2 = ps2.tile([128, 192], dt.float32, tag="ps2")
                    rhsA = stv2[:, clsA, 0:192]   # [128, 192] tiles k=0..5
                    rhsB = stv2[:, clsB, 32:224]  # [128, 192] tiles k=1..6
                    nc.tensor.matmul(
                        p2[0:81, :],
                        selb[:, clsA * 81 : (clsA + 1) * 81],
                        rhsA,
                        start=True,
                        stop=False,
                    )
                    nc.tensor.matmul(
                        p2[0:81, :],
                        selb[:, clsB * 81 : (clsB + 1) * 81],
                        rhsB,
                        start=False,
                        stop=True,
                    )
                    dst = outt[:, wl * 192 : (wl + 1) * 192]
                    src = p2[0:81, :]
                    if eng == 0:
                        nc.vector.tensor_copy(dst, src)
                    else:
                        nc.scalar.copy(dst, src)
                    eng ^= 1
                # store chunk (q, tt) on the gpsimd SWDGE queue
                chunk = (q * 8 + (t - 8)) * 1536
                nc.gpsimd.dma_start(
                    out[:, chunk : chunk + 1536], outt[:, :]
                )


def _build_nc():
    import concourse.mybir as mybir
    import concourse.tile as tile
    from concourse import bacc

    dt = mybir.dt
    nc = bacc.Bacc("TRN2", target_bir_lowering=False, debug=False)
    f1 = nc.dram_tensor("f1", [C, OH * W], dt.float16, kind="ExternalInput")
    f2 = nc.dram_tensor(
        "f2", [C, NSLAB * F2W * 8], dt.float16, kind="ExternalInput"
    )
    selt = nc.dram_tensor("sel", [C, 128 * 81], dt.float16, kind="ExternalInput")
    out = nc.dram_tensor("out", [D, 2 * 8 * 1536], dt.float16, kind="ExternalOutput")
    with tile.TileContext(nc) as tc:
        _emit(tc, f1[:, :], f2[:, :], selt[:, :], out[:, :])
    nc.finalize()
    return nc


def _shard_inputs(feat1, feat2):
    sel = _build_sel()
    in_maps = []
    for core in range(NCORES):
        b, half = core // 2, core % 2
        h0 = half * OH
        f1s = np.ascontiguousarray(
            (feat1[b, :, h0 : h0 + OH, :] * SCALE).astype(np.float16).reshape(C, OH * W)
        )
        f2pad = np.zeros((C, F2H, F2W), np.float16)
        lo, hi = h0 - 4, h0 + OH + 4
        slo, shi = max(lo, 0), min(hi, H)
        f2pad[:, slo - lo : shi - lo, 4 : 4 + W] = feat2[b, :, slo:shi, :].astype(
            np.float16
        )
        # slab s -> f2pad rows [8s, 8s+8), transposed to [C, w, r] so each
        # (16 w x 8 r) matmul weight window is contiguous.
        slabs = np.zeros((C, NSLAB, F2W, 8), np.float16)
        for s in range(NSLAB):
            slabs[:, s] = f2pad[:, 8 * s : 8 * s + 8, :].transpose(0, 2, 1)
        in_maps.append(
            {
                "f1": f1s,
                "f2": np.ascontiguousarray(slabs.reshape(C, -1)),
                "sel": sel,
            }
        )
    return in_maps


def _unshard(results):
    """results: per-core dicts with 'out' [81, 2*8*1536] f16 ->
    full [B, D, H, W] f32."""
    full = np.zeros((B, D, H, W), np.float32)
    for core in range(NCORES):
        b, half = core // 2, core % 2
        o = results[core]["out"].reshape(D, 2, 8, 8, 6, 32)  # d,q,tt,wl,k,w0
        o = o.transpose(0, 1, 4, 2, 5, 3).reshape(D, 2, HQ, W)  # d,q,(k tt),(w0 wl)
        full[b, :, half * OH : (half + 1) * OH, :] = o.reshape(
            D, OH, W
        ).astype(np.float32)
    return full


def kernel(feat1, feat2):
    feat1 = np.asarray(feat1, dtype=np.float32)
    feat2 = np.asarray(feat2, dtype=np.float32)
    from concourse.bass_utils import run_bass_kernel_spmd

    nc = _build_nc()
    in_maps = _shard_inputs(feat1, feat2)
    res = run_bass_kernel_spmd(nc, in_maps, list(range(NCORES)))
    return _unshard(res.results)


# revision 4
# speedup vs baseline: 1.1516x; 1.0955x over previous
"""Cost-volume builder (correlation layer) for Trainium2, 8-core SPMD.

out[b, d, h, w] = (1/sqrt(C)) * sum_c feat1[b,c,h,w] * feat2[b,c,h+dy,w+dx]
for d = (dy+4)*9 + (dx+4), dy,dx in [-4,4]. B,C,H,W = 4,128,192,256.

Sharding: 8 cores = 4 batches x 2 H-halves (96 rows each, feat2 halo +-4).

Per-core algorithm (two 48-row q-halves sharing one f1 buffer):
  Pass 1 (PE): per r-block (8 feat2 rows) x w-tile: two column-tiled
    matmuls (64 weight cols each, parallel LDWEIGHTS) of
    lhsT=F2win[C,8x16] vs rhs=F1[C,16hx8w] -> band tile [128,128] PSUM:
    band[(j,we), (h,w)] = sum_c F2[c,r0+j,we] * F1[c,h,w].
    Edge r-blocks (q0k0 / q1k6) clip the rhs to the valid 8 h rows.
  Stage (DVE/ACT): PSUM->SBUF fp16 cast copies; half-copies where only
    one cls-half is ever read (k=0 / k=6 of each half).
  Pass 2 (PE): 128 one-hot selection matrices Sel[128,81]; per (t, wl)
    two PSUM-accumulated matmuls over paired r-blocks gather each output
    position's 81 displacement values.
  Out-copies (DVE/ACT): [81,192] PSUM->SBUF fp16, chunk-DMA per (q,t);
    host reassembles (q,tt,wl,k,w0) order and casts fp32.
"""

import math

import numpy as np

B, C, H, W = 4, 128, 192, 256
D = 81
NCORES = 8
OH = H // 2            # 96 output rows per core
HQ = 48                # rows per processed half
NRB = 7                # r-blocks per half, 8 rows each
NSLAB = 13             # unique f2 slabs per core (slab 6 shared by halves)
WT = 32                # w-tiles per row (T=8)
T = 8
WE = 16                # feat2 w-window per tile
F2W = W + 8            # 264, zero-padded W
F2H = OH + 8           # 104 rows incl halo
SCALE = 1.0 / math.sqrt(C)


def _build_sel():
    """[128, 128*81] fp16 one-hot selection matrices, class c=(h_off*8+wl).
    Weight-column order of pass-1 lhsT is (kappa, j): row = kappa*8 + j."""
    sel = np.zeros((128, 128, 81), np.float16)
    for h_off in range(16):
        for wl in range(8):
            cls = h_off * 8 + wl
            for j in range(8):
                dy = j + 4 - h_off
                if -4 <= dy <= 4:
                    for dxh in range(9):  # dxh = dx + 4
                        row = (wl + dxh) * 8 + j
                        col = (dy + 4) * 9 + dxh
                        sel[row, cls, col] = 1.0
    return sel.reshape(128, 128 * 81)


def _emit(tc, f1, f2, selt, out):
    """Emit the Tile program. f1:[C,96*W] f16, f2:[C,13*F2W*8] f16,
    selt:[C,128*81] f16, out:[D,2*8*1536] f16 (DRAM APs)."""
    import concourse.bass as bass
    import concourse.mybir as mybir

    dt = mybir.dt
    nc = tc.nc
    MS = bass.MemorySpace

    with (
        tc.tile_pool(name="const", bufs=1) as cpool,
        tc.tile_pool(name="f1p", bufs=1) as f1p,
        tc.tile_pool(name="f2p", bufs=4) as f2p,
        tc.tile_pool(name="stgp", bufs=1) as stgp,
        tc.tile_pool(name="outp", bufs=2) as outp,
        tc.tile_pool(name="ps1", bufs=4, space=MS.PSUM) as ps1,
        tc.tile_pool(name="ps2", bufs=4, space=MS.PSUM) as ps2,
    ):
        # ---- persistent tiles ----
        f1buf = f1p.tile([128, OH * W], dt.float16)     # rows h in [0, 96)
        selb = cpool.tile([128, 128 * 81], dt.float16)
        slab6 = cpool.tile([128, F2W * 8], dt.float16)

        # ---- load schedule: alternate the two HWDGE rings (sync/scalar),
        # criticality order: slab0, f1c0, slab1, f1c1, ... sel mid-stream.
        slab_tiles = {}

        def load_slab(s):
            if s == 6:
                tile_ = slab6
            else:
                tile_ = f2p.tile([128, F2W * 8], dt.float16, tag="f2s")
            eng = nc.sync if (s % 2 == 0) else nc.scalar
            eng.dma_start(tile_[:, :], f2[:, s * F2W * 8 : (s + 1) * F2W * 8])
            slab_tiles[s] = tile_

        # f1 in 6 chunks of 16 rows
        def load_f1(c):
            eng = nc.scalar if (c % 2 == 0) else nc.sync
            eng.dma_start(
                f1buf[:, 16 * c * W : (16 * c + 16) * W],
                f1[:, 16 * c * W : (16 * c + 16) * W],
            )

        load_slab(0)
        load_f1(0)
        load_slab(1)
        load_f1(1)
        load_slab(2)
        load_f1(2)
        load_slab(3)
        load_f1(3)
        load_slab(4)
        nc.scalar.dma_start(selb[:, :], selt[:, :])
        load_f1(4)
        load_slab(5)
        load_f1(5)
        load_slab(6)

        f1v = f1buf[:, :].rearrange("p (h x) -> p h x", h=OH)
        eng = 0

        for q in range(2):
            # stage layout: col = cls * 224 + (k*32 + w0), cls = h_off*8 + wl
            stg = stgp.tile([128, 128 * NRB * WT], dt.float16, tag="stg")
            stv2 = stg[:, :].rearrange("p (c t) -> p c t", c=128)

            # ---- pass 1: band matmuls (column-tiled 2x64) ----
            for k in range(NRB):
                s = 6 * q + k
                if s not in slab_tiles:
                    load_slab(s)
                f2s = slab_tiles[s]
                r0 = 48 * q + 8 * k - 8
                lo = max(r0, 0)          # clipped rhs rows [lo, hi)
                hi = min(r0 + 16, OH)
                n = (hi - lo) * T        # rhs cols per w-tile (128 or 64)
                c0 = (lo - r0) * 8       # first psum col (cls offset)
                for g in range(8):  # groups of 4 w-tiles per PSUM bank
                    pt = ps1.tile([128, 4 * n], dt.float32, tag="ps1")
                    for u in range(4):
                        w0 = g * 4 + u
                        rhs = f1v[:, lo:hi, 8 * w0 : 8 * w0 + T]
                        for cg in range(2):  # column-tiled halves
                            lhsT = f2s[
                                :, 64 * w0 + 64 * cg : 64 * w0 + 64 * cg + 64
                            ]
                            nc.tensor.matmul(
                                pt[64 * cg : 64 * cg + 64, u * n : (u + 1) * n],
                                lhsT,
                                rhs,
                                start=True,
                                stop=True,
                                tile_position=(0, 64 * cg),
                            )
                    # psum col = u*n + (cls - c0)  ->  stage (cls, t0+u)
                    t0 = k * 32 + g * 4
                    src = pt[:, :].rearrange("p (u c) -> p c u", u=4)
                    # skip never-read cls halves at the seam r-blocks
                    if q == 0 and k == 6:
                        src, cl0, cl1 = src[:, 0:64, :], 0, 64
                    elif q == 1 and k == 0:
                        src, cl0, cl1 = src[:, 64:128, :], 64, 128
                    else:
                        cl0, cl1 = c0, c0 + n
                    dst = stv2[:, cl0:cl1, t0 : t0 + 4]
                    if eng == 0:
                        nc.vector.tensor_copy(dst, src)
                    else:
                        nc.scalar.copy(dst, src)
                    eng ^= 1

            # ---- pass 2: selection matmuls + out copies + chunked stores ----
            for t in range(8, 16):
                outt = outp.tile([81, 8 * 192], dt.float16, tag="outt")
                for wl in range(8):
                    clsA = t * 8 + wl
                    clsB = (t - 8) * 8 + wl
                    p2 = ps2.tile([128, 192], dt.float32, tag="ps2")
                    rhsA = stv2[:, clsA, 0:192]   # [128, 192] tiles k=0..5
                    rhsB = stv2[:, clsB, 32:224]  # [128, 192] tiles k=1..6
                    nc.tensor.matmul(
                        p2[0:81, :],
                        selb[:, clsA * 81 : (clsA + 1) * 81],
                        rhsA,
                        start=True,
                        stop=False,
                    )
                    nc.tensor.matmul(
                        p2[0:81, :],
                        selb[:, clsB * 81 : (clsB + 1) * 81],
                        rhsB,
                        start=False,
                        stop=True,
                    )
                    dst = outt[:, wl * 192 : (wl + 1) * 192]
                    src = p2[0:81, :]
                    if eng == 0:
                        nc.vector.tensor_copy(dst, src)
                    else:
                        nc.scalar.copy(dst, src)
                    eng ^= 1
                # store chunk (q, tt) on the gpsimd SWDGE queue
                chunk = (q * 8 + (t - 8)) * 1536
                nc.gpsimd.dma_start(
                    out[:, chunk : chunk + 1536], outt[:, :]
                )


def _build_nc():
    import concourse.mybir as mybir
    import concourse.tile as tile
    from concourse import bacc

    dt = mybir.dt
    nc = bacc.Bacc("TRN2", target_bir_lowering=False, debug=False)
    f1 = nc.dram_tensor("f1", [C, OH * W], dt.float16, kind="ExternalInput")
    f2 = nc.dram_tensor(
        "f2", [C, NSLAB * F2W * 8], dt.float16, kind="ExternalInput"
    )
    selt = nc.dram_tensor("sel", [C, 128 * 81], dt.float16, kind="ExternalInput")
    out = nc.dram_tensor("out", [D, 2 * 8 * 1536], dt.float16, kind="ExternalOutput")
    with tile.TileContext(nc) as tc:
        _emit(tc, f1[:, :], f2[:, :], selt[:, :], out[:, :])
    nc.finalize()
    return nc


def _shard_inputs(feat1, feat2):
    sel = _build_sel()
    in_maps = []
    for core in range(NCORES):
        b, half = core // 2, core % 2
        h0 = half * OH
        f1s = np.ascontiguousarray(
            (feat1[b, :, h0 : h0 + OH, :] * SCALE).astype(np.float16).reshape(C, OH * W)
        )
        f2pad = np.zeros((C, F2H, F2W), np.float16)
        lo, hi = h0 - 4, h0 + OH + 4
        slo, shi = max(lo, 0), min(hi, H)
        f2pad[:, slo - lo : shi - lo, 4 : 4 + W] = feat2[b, :, slo:shi, :].astype(
            np.float16
        )
        # slab s -> f2pad rows [8s, 8s+8), transposed to [C, w, r] so each
        # (16 w x 8 r) matmul weight window is contiguous.
        slabs = np.zeros((C, NSLAB, F2W, 8), np.float16)
        for s in range(NSLAB):
            slabs[:, s] = f2pad[:, 8 * s : 8 * s + 8, :].transpose(0, 2, 1)
        in_maps.append(
            {
                "f1": f1s,
                "f2": np.ascontiguousarray(slabs.reshape(C, -1)),
                "sel": sel,
            }
        )
    return in_maps


def _unshard(results):
    """results: per-core dicts with 'out' [81, 2*8*1536] f16 ->
    full [B, D, H, W] f32."""
    full = np.zeros((B, D, H, W), np.float32)
    for core in range(NCORES):
        b, half = core // 2, core % 2
        o = results[core]["out"].reshape(D, 2, 8, 8, 6, 32)  # d,q,tt,wl,k,w0
        o = o.transpose(0, 1, 4, 2, 5, 3).reshape(D, 2, HQ, W)  # d,q,(k tt),(w0 wl)
        full[b, :, half * OH : (half + 1) * OH, :] = o.reshape(
            D, OH, W
        ).astype(np.float32)
    return full


def kernel(feat1, feat2):
    feat1 = np.asarray(feat1, dtype=np.float32)
    feat2 = np.asarray(feat2, dtype=np.float32)
    from concourse.bass_utils import run_bass_kernel_spmd

    nc = _build_nc()
    in_maps = _shard_inputs(feat1, feat2)
    res = run_bass_kernel_spmd(nc, in_maps, list(range(NCORES)))
    return _unshard(res.results)
